# revision 5
# baseline (speedup 1.0000x reference)
"""BoundaryLoss Trainium2 kernel (8 NeuronCores, data-parallel over batch).

Per core (one (21,512,512) image): ce[p] = ln(sum_c exp(x[c,p])) - x[t[p],p],
weighted by w[p] = 1 + 2*boundary[p] and summed; host sums 8 partials / BHW.

v2 layout/engine plan (from the v1 trace: DVE 86us busy — u8 stt runs 1x; the
AllReduce spanned 16->110us; ACT 60us):
- All t data is bf16 (host-cast), so every DVE op gets its 2x/4x perf mode.
  The gather mask is ts(tb16==cvec) [4x] then tt(mk, mk, x) [2x] instead of
  the 1x-only scalar_tensor_tensor.
- Pixels = 32 superblocks x 8192.  Chunks 0-4 pack 4 channels x 32 sb onto
  128 partitions (contiguous 2MB loads, 16KB descriptors); sums/gath reduce
  via block-ones kxm into the 4 PSUM windows (quadrant tile_position).
  Channel 20 instead uses the flat [128,2048] layout with a permutation
  stationary writing all 128 PSUM partitions in 4 matmuls — so its exp/mask
  cost 2048 cols, not 8192.
- Boundary map from flat bf16 t at +-512 offsets, 3-tap ORs, borders zeroed,
  then the (512,512) bf16 AllReduce(add) is triggered ~10us into the kernel
  so it finishes well before the tail needs it.
- Final: ln(sums)->bf16 in halves; two scalar_tensor_tensor ops per half with
  accum_out produce sum(w*lnS) and sum(-w*gath) directly (w/-w images from
  the reduced map); ones-matmul + reduce + scaled copy -> out.
"""

import sys

sys.path.insert(0, "/opt/trn_rl_repo")

import numpy as np
import ml_dtypes

import concourse.bass as bass
import concourse.bacc as bacc
import concourse.tile as tile
from concourse import mybir
from concourse import bass_utils

F32 = mybir.dt.float32
BF16 = mybir.dt.bfloat16

C = 21          # channels
H = W = 512
NPIX = H * W    # 262144 pixels per core
NCORES = 8
NTOT = float(NCORES * NPIX)

Exp = mybir.ActivationFunctionType.Exp
Ln = mybir.ActivationFunctionType.Ln
Copy = mybir.ActivationFunctionType.Copy
op = mybir.AluOpType


def _consts():
    # kxm[p, m] = 1 if p % 32 == m: block-sum over the 4 channels packed per
    # chunk (partition p = c_local*32 + superblock).
    kxm = np.zeros((128, 32), np.float32)
    for p in range(128):
        kxm[p, p % 32] = 1.0
    # perm[p, m] = 1 iff m = 32*(p%4) + p//4: maps the flat-layout partition
    # p = sb*4 + w of channel 20 onto PSUM row 32*w + sb.
    perm = np.zeros((128, 128), np.float32)
    for p in range(128):
        perm[p, 32 * (p % 4) + p // 4] = 1.0
    # cvec[p, k] = absolute channel index of partition p in chunk k.
    cvec = np.zeros((128, 5), np.float32)
    for k in range(5):
        cvec[:, k] = 4 * k + np.arange(128) // 32
    return (
        kxm.astype(ml_dtypes.bfloat16),
        perm.astype(ml_dtypes.bfloat16),
        cvec,
    )


def build_nc(use_cc=True):
    nc = bacc.Bacc(
        "TRN2",
        target_bir_lowering=False,
        debug=False,
        num_devices=NCORES,
        num_swdge_queues=1,
        dynamic_dma_scratch_size=16384,
    )

    x_d = nc.dram_tensor("x", [C, NPIX], BF16, kind="ExternalInput")
    t_d = nc.dram_tensor("t16", [H, W], BF16, kind="ExternalInput")
    out_d = nc.dram_tensor("out", [1, 1], F32, kind="ExternalOutput")

    kxm_np, perm_np, cvec_np = _consts()
    kxm_d = nc.inline_tensor(kxm_np, name="kxm")
    perm_d = nc.inline_tensor(perm_np, name="perm")
    cvec_d = nc.inline_tensor(cvec_np, name="cvec")
    ones_d = nc.inline_tensor(np.ones((128, 1), np.float32), name="ones")

    groups = [list(range(NCORES))]

    with tile.TileContext(nc) as tc:
        with (
            tc.tile_pool(name="singles", bufs=1) as singles,
            tc.tile_pool(name="xpool", bufs=4) as xpool,
            tc.tile_pool(name="expool", bufs=2) as expool,
            tc.tile_pool(name="mkpool", bufs=2) as mkpool,
            tc.tile_pool(name="bm", bufs=1) as bm,
            tc.tile_pool(name="psum", bufs=1, space="PSUM") as psum,
            tc.tile_pool(name="dram", bufs=1, space="DRAM") as dram,
        ):
            # ---- consts to SBUF (sync/HWDGE queue) ----
            kxm = singles.tile([128, 32], BF16, tag="kxm")
            perm = singles.tile([128, 128], BF16, tag="perm")
            cvec = singles.tile([128, 5], F32, tag="cvec")
            ones = singles.tile([128, 1], F32, tag="ones")
            nc.sync.dma_start(kxm[:], kxm_d[:])
            nc.sync.dma_start(perm[:], perm_d[:])
            nc.sync.dma_start(cvec[:], cvec_d[:])
            nc.sync.dma_start(ones[:], ones_d[:])

            # ---- t loads (flat [128,2048] bf16 image at offsets 0/+512/-512)
            tflat = t_d.ap().rearrange("h w -> (h w)")
            tden = bm.tile([128, 2048], BF16, tag="tden")
            tsh = bm.tile([128, 2048], BF16, tag="tsh")
            tshm = bm.tile([128, 2048], BF16, tag="tshm")
            zrow = singles.tile([1, W], BF16, tag="zrow")
            nc.vector.memset(zrow[:], 0.0)
            nc.sync.dma_start(tsh[127:128, 1536:2048], zrow[:])
            nc.vector.memset(tshm[0:1, 0:512], 0)
            nc.sync.dma_start(tden[:], tflat.rearrange("(P f) -> P f", P=128))
            nc.sync.dma_start(
                tsh[0:127, :],
                tflat[512 : 512 + 127 * 2048].rearrange("(P f) -> P f", P=127),
            )
            nc.sync.dma_start(tsh[127:128, 0:1536], tflat[260608:262144][None, :])
            nc.sync.dma_start(tshm[0:1, 512:2048], tflat[0:1536][None, :])
            nc.sync.dma_start(
                tshm[1:128, :],
                tflat[1536 : 1536 + 127 * 2048].rearrange("(P f) -> P f", P=127),
            )

            # ---- x views ----
            xv = x_d.ap().rearrange("c (B n) -> c B n", n=8192)  # (21,32,8192)
            tvs = t_d.ap().rearrange("(B r) w -> B (r w)", r=16)  # (32,8192)

            # ---- gpsimd/SWDGE queue: x chunk loads ----
            x_tiles = []
            x0 = xpool.tile([128, 8192], BF16, tag="x")
            for q in range(4):
                nc.gpsimd.dma_start(
                    x0[:, 2048 * q : 2048 * (q + 1)],
                    xv[0:4, :, 2048 * q : 2048 * (q + 1)],
                )
            x_tiles.append(x0)
            tb16 = singles.tile([128, 8192], BF16, tag="tb16")
            nc.gpsimd.dma_start(tb16[:], tvs[None, :, :].to_broadcast((4, 32, 8192)))
            for k in (1, 2):
                xk = xpool.tile([128, 8192], BF16, tag="x")
                nc.gpsimd.dma_start(xk[:], xv[4 * k : 4 * k + 4, :, :])
                x_tiles.append(xk)

            # ---- boundary map on DVE (while x0 is in flight) ----
            # rd = (tden != tsh); rdm = (tshm != tden); dv = rd|rdm -> rd;
            # 3-tap horizontal OR -> rdm; zero borders of rdm.
            rd = bm.tile([128, 2048], BF16, tag="rd")
            rdm = bm.tile([128, 2048], BF16, tag="rdm")
            nc.vector.tensor_tensor(rd[:], tden[:], tsh[:], op.not_equal)
            nc.vector.tensor_tensor(rdm[:], tshm[:], tden[:], op.not_equal)
            nc.vector.tensor_tensor(rd[:], rd[:], rdm[:], op.max)
            nc.vector.tensor_tensor(
                rdm[:, 1:2047], rd[:, 0:2046], rd[:, 1:2047], op.max
            )
            nc.vector.tensor_tensor(
                rdm[:, 1:2047], rdm[:, 1:2047], rd[:, 2:2048], op.max
            )
            rv = rdm[:].rearrange("P (r w) -> P r w", w=W)
            nc.vector.memset(rv[:, :, 0:1], 0.0)
            nc.vector.memset(rv[:, :, 511:512], 0.0)
            nc.vector.memset(rdm[0:1, 0:W], 0.0)
            nc.sync.dma_start(rdm[127:128, 3 * W : 4 * W], zrow[:])

            # ---- collective: AllReduce(add) of the local map ----
            cc_in = dram.tile([H, W], BF16, tag="cc_in")
            cc_out = dram.tile([H, W], BF16, tag="cc_out")
            nc.sync.dma_start(
                cc_in[:].rearrange("(P r) w -> P (r w)", r=4), rdm[:]
            )
            if use_cc:
                nc.gpsimd.collective_compute(
                    "AllReduce",
                    op.add,
                    replica_groups=groups,
                    ins=[cc_in.opt()],
                    outs=[cc_out.opt()],
                )
            else:
                cc_out = cc_in

            # remaining x loads (queued behind the cc trigger; the trigger
            # fires as soon as cc_in lands, ~10us in)
            for k in (3, 4):
                xk = xpool.tile([128, 8192], BF16, tag="x")
                nc.gpsimd.dma_start(xk[:], xv[4 * k : 4 * k + 4, :, :])
                x_tiles.append(xk)
            x21 = singles.tile([128, 2048], BF16, tag="x21")
            nc.gpsimd.dma_start(
                x21[:], x_d.ap()[20:21, :].rearrange("c (P n) -> (c P) n", n=2048)
            )

            # ---- weight image from the reduced map (bd -> w16 / wneg16) ----
            bd = singles.tile([128, 2048], BF16, tag="bd")
            ccv = (
                cc_out[:]
                .rearrange("(B r) w -> B (r w)", r=16)
                .rearrange("B (q n) -> B q n", q=4)
            )
            for q in range(4):
                nc.sync.dma_start(bd[32 * q : 32 * q + 32, :], ccv[:, q, :])

            # ---- main loop: 5 chunks of 4 channels ----
            sums = psum.tile([128, 2048], F32, tag="sums")
            gath = psum.tile([128, 2048], F32, tag="gath")
            for k in range(5):
                x_t = x_tiles[k]
                ex = expool.tile([128, 8192], BF16, tag="ex")
                mk = mkpool.tile([128, 8192], BF16, tag="mk")
                npc = 4 if k == 0 else 1
                fpp = 8192 // npc
                for h in range(npc):
                    sl = slice(fpp * h, fpp * (h + 1))
                    nc.scalar.activation(ex[:, sl], x_t[:, sl], Exp)
                    nc.vector.tensor_scalar(
                        mk[:, sl], tb16[:, sl], cvec[:, k : k + 1], None, op.is_equal
                    )
                    nc.vector.tensor_tensor(mk[:, sl], mk[:, sl], x_t[:, sl], op.mult)
                    for wi in range(fpp // 2048):
                        w4 = (fpp // 2048) * h + wi
                        q0 = 32 * w4
                        for j in range(4):
                            fs = 2048 * w4 + 512 * j
                            nc.tensor.matmul(
                                sums[q0 : q0 + 32, 512 * j : 512 * (j + 1)],
                                kxm[:, :],
                                ex[:, fs : fs + 512],
                                start=(k == 0),
                                stop=False,
                                tile_position=(0, q0),
                                skip_group_check=True,
                            )
                            nc.tensor.matmul(
                                gath[q0 : q0 + 32, 512 * j : 512 * (j + 1)],
                                kxm[:, :],
                                mk[:, fs : fs + 512],
                                start=(k == 0),
                                stop=False,
                                tile_position=(0, q0),
                                skip_group_check=True,
                            )

            # ---- channel 20 (flat [128,2048] layout, permutation stationary)
            ex21 = singles.tile([128, 2048], BF16, tag="ex21")
            mk21 = singles.tile([128, 2048], BF16, tag="mk21")
            nc.scalar.activation(ex21[:], x21[:], Exp)
            nc.vector.tensor_scalar(mk21[:], tden[:], 20.0, None, op.is_equal)
            nc.vector.tensor_tensor(mk21[:], mk21[:], x21[:], op.mult)

            # w / -w images (bd depends on the collective; emitted here so the
            # DVE only stalls on it right before the final phase)
            w16 = singles.tile([128, 2048], BF16, tag="w16")
            wneg = singles.tile([128, 2048], BF16, tag="wneg")
            nc.vector.tensor_scalar(w16[:], bd[:], 0.0, None, op.is_gt)
            nc.vector.tensor_scalar(wneg[:], w16[:], -2.0, -1.0, op.mult, op.add)
            nc.vector.tensor_scalar(w16[:], w16[:], 2.0, 1.0, op.mult, op.add)

            logs = singles.tile([128, 2048], BF16, tag="logs")
            partials = singles.tile([128, 4], F32, tag="partials")
            # last-chunk matmuls and the final phase, interleaved per half so
            # ln/stt of banks 0-1 overlap the matmuls of banks 2-3
            for half in range(2):
                js = (0, 1) if half == 0 else (2, 3)
                for j in js:
                    nc.tensor.matmul(
                        sums[:, 512 * j : 512 * (j + 1)],
                        perm[:, :],
                        ex21[:, 512 * j : 512 * (j + 1)],
                        start=False,
                        stop=True,
                        tile_position=(0, 0),
                        skip_group_check=True,
                    )
                for j in js:
                    nc.tensor.matmul(
                        gath[:, 512 * j : 512 * (j + 1)],
                        perm[:, :],
                        mk21[:, 512 * j : 512 * (j + 1)],
                        start=False,
                        stop=True,
                        tile_position=(0, 0),
                        skip_group_check=True,
                    )
                hs = slice(1024 * half, 1024 * (half + 1))
                nc.scalar.activation(logs[:, hs], sums[:, hs], Ln)
                wd = singles.tile([128, 1024], BF16, tag=f"wd{half}")
                wd2 = singles.tile([128, 1024], BF16, tag=f"wd2{half}")
                nc.vector.scalar_tensor_tensor(
                    wd[:], logs[:, hs], 1.0, w16[:, hs], op.mult, op.mult,
                    accum_out=partials[:, 2 * half : 2 * half + 1],
                )
                nc.vector.scalar_tensor_tensor(
                    wd2[:], gath[:, hs], 1.0, wneg[:, hs], op.mult, op.mult,
                    accum_out=partials[:, 2 * half + 1 : 2 * half + 2],
                )

            # ---- final reduction ----
            totp = psum.tile([1, 4], F32, tag="sums")
            nc.tensor.matmul(totp[:], ones[:], partials[:], start=True, stop=True)
            finr = singles.tile([1, 1], F32, tag="finr")
            nc.vector.reduce_sum(finr[:], totp[:], axis=mybir.AxisListType.X)
            fin = singles.tile([1, 1], F32, tag="fin")
            nc.scalar.activation(fin[:], finr[:], Copy, scale=1.0 / NTOT)
            nc.sync.dma_start(out_d[:], fin[:])

    nc.compile()
    return nc


_NC = None


def _get_nc():
    global _NC
    if _NC is None:
        _NC = build_nc()
    return _NC


def make_in_maps(inputs, targets):
    in_maps = []
    for i in range(NCORES):
        in_maps.append(
            {
                "x": np.ascontiguousarray(
                    np.asarray(inputs[i], dtype=np.float32)
                    .reshape(C, NPIX)
                    .astype(ml_dtypes.bfloat16)
                ),
                "t16": np.asarray(targets[i]).astype(ml_dtypes.bfloat16),
            }
        )
    return in_maps


def run_device(inputs, targets, trace=False):
    nc = _get_nc()
    res = bass_utils.run_bass_kernel_spmd(
        nc,
        make_in_maps(inputs, targets),
        core_ids=list(range(NCORES)),
        trace=trace,
    )
    return res


def kernel(inputs, targets):
    res = run_device(inputs, targets, trace=False)
    # each core returns its local weighted-sum / (B*H*W); the global mean is
    # the sum of the 8 partials (final reduction of the batch shard).
    return np.float32(sum(float(r["out"][0, 0]) for r in res.results))


# revision 6
# speedup vs baseline: 1.3558x; 1.3558x over previous
"""BoundaryLoss Trainium2 kernel (8 NeuronCores, data-parallel over batch).

Per core (one (21,512,512) image): ce[p] = ln(sum_c exp(x[c,p])) - x[t[p],p],
weighted by w[p] = 1 + 2*boundary[p] and summed; host sums 8 partials / BHW.

v3 layout/engine plan (from the v1/v2 traces):
- All t data is bf16 (host-cast) so every DVE op gets its 2x/4x perf mode:
  the gather mask is ts(tb16==cvec) [4x] then tt(mk, mk, x) [2x] instead of
  the 1x-only u8 scalar_tensor_tensor (v1: 65us of DVE).
- Pixels = 32 superblocks x 8192.  Chunks 0-4 pack 4 channels x 32 sb onto
  128 partitions (contiguous 2MB loads, 16KB descriptors); sums/gath reduce
  via block-ones kxm into the 4 PSUM windows (quadrant tile_position).
  Channel 20 instead uses the flat [128,2048] layout with a permutation
  stationary writing all 128 PSUM partitions in 4 matmuls, so its exp/mask
  cost 2048 free cols, not 8192.
- ALL dma doorbells (t loads first, then every x chunk; x pool bufs=5 so no
  buffer-reuse wait blocks the queue) are issued on the gpsimd/SWDGE queue
  before the collective trigger: v2 showed a cc trigger mid-queue stalls
  every later doorbell until the collective completes.  Small/async traffic
  (consts, zero-rows, cc_in store, bd, out) rides the sync/HWDGE ring.
- Boundary map from flat bf16 t at +-512 offsets (3-tap vertical any-diff is
  elementwise per partition), horizontal taps via free-shifts, borders
  zeroed, packed to fp8 (256KB) and AllReduce(add) triggered ~10us in so it
  completes well before the tail needs it.
- Final: ln(sums) in halves; two scalar_tensor_tensor ops per half with
  accum_out produce sum(w*lnS) and sum(-w*gath) directly (w/-w images from
  the reduced map); ones-matmul + reduce + scaled copy -> out.
"""

import sys

sys.path.insert(0, "/opt/trn_rl_repo")

import numpy as np
import ml_dtypes

import concourse.bass as bass
import concourse.bacc as bacc
import concourse.tile as tile
from concourse import mybir
from concourse import bass_utils

F32 = mybir.dt.float32
BF16 = mybir.dt.bfloat16
FP8 = mybir.dt.float8e4

C = 21          # channels
H = W = 512
NPIX = H * W    # 262144 pixels per core
NCORES = 8
NTOT = float(NCORES * NPIX)

Exp = mybir.ActivationFunctionType.Exp
Ln = mybir.ActivationFunctionType.Ln
Copy = mybir.ActivationFunctionType.Copy
op = mybir.AluOpType


def _consts():
    # kxm[p, m] = 1 if p % 32 == m: block-sum over the 4 channels packed per
    # chunk (partition p = c_local*32 + superblock).
    kxm = np.zeros((128, 32), np.float32)
    for p in range(128):
        kxm[p, p % 32] = 1.0
    # perm[p, m] = 1 iff m = 32*(p%4) + p//4: maps the flat-layout partition
    # p = sb*4 + w of channel 20 onto PSUM row 32*w + sb.
    perm = np.zeros((128, 128), np.float32)
    for p in range(128):
        perm[p, 32 * (p % 4) + p // 4] = 1.0
    # cvec[p, k] = absolute channel index of partition p in chunk k.
    cvec = np.zeros((128, 5), np.float32)
    for k in range(5):
        cvec[:, k] = 4 * k + np.arange(128) // 32
    return (
        kxm.astype(ml_dtypes.bfloat16),
        perm.astype(ml_dtypes.bfloat16),
        cvec,
    )


def build_nc(use_cc=True):
    nc = bacc.Bacc(
        "TRN2",
        target_bir_lowering=False,
        debug=False,
        num_devices=NCORES,
        num_swdge_queues=1,
        dynamic_dma_scratch_size=16384,
    )

    x_d = nc.dram_tensor("x", [C, NPIX], BF16, kind="ExternalInput")
    t_d = nc.dram_tensor("t16", [H, W], BF16, kind="ExternalInput")
    out_d = nc.dram_tensor("out", [1, 1], F32, kind="ExternalOutput")

    kxm_np, perm_np, cvec_np = _consts()
    kxm_d = nc.inline_tensor(kxm_np, name="kxm")
    perm_d = nc.inline_tensor(perm_np, name="perm")
    cvec_d = nc.inline_tensor(cvec_np, name="cvec")
    ones_d = nc.inline_tensor(np.ones((128, 1), np.float32), name="ones")

    groups = [list(range(NCORES))]

    with tile.TileContext(nc) as tc:
        with (
            tc.tile_pool(name="singles", bufs=1) as singles,
            tc.tile_pool(name="xpool", bufs=5) as xpool,
            tc.tile_pool(name="expool", bufs=2) as expool,
            tc.tile_pool(name="mkpool", bufs=2) as mkpool,
            tc.tile_pool(name="bm", bufs=1) as bm,
            tc.tile_pool(name="psum", bufs=1, space="PSUM") as psum,
            tc.tile_pool(name="dram", bufs=1, space="DRAM") as dram,
        ):
            # ---- consts to SBUF (sync/HWDGE queue) ----
            kxm = singles.tile([128, 32], BF16, tag="kxm")
            perm = singles.tile([128, 128], BF16, tag="perm")
            cvec = singles.tile([128, 5], F32, tag="cvec")
            ones = singles.tile([128, 1], F32, tag="ones")
            nc.sync.dma_start(kxm[:], kxm_d[:])
            nc.sync.dma_start(perm[:], perm_d[:])
            nc.sync.dma_start(cvec[:], cvec_d[:])
            nc.sync.dma_start(ones[:], ones_d[:])

            # ---- t loads (flat [128,2048] bf16 image at offsets 0/+512/-512)
            # on the gpsimd queue ahead of x: v2 showed the 3KB tail pieces
            # starve for ~60us on the sync ring under collective traffic.
            tflat = t_d.ap().rearrange("h w -> (h w)")
            tden = bm.tile([128, 2048], BF16, tag="tden")
            tsh = bm.tile([128, 2048], BF16, tag="tsh")
            tshm = bm.tile([128, 2048], BF16, tag="tshm")
            zrow = singles.tile([1, W], BF16, tag="zrow")
            nc.vector.memset(zrow[:], 0.0)
            nc.sync.dma_start(tsh[127:128, 1536:2048], zrow[:])
            nc.vector.memset(tshm[0:1, 0:512], 0)
            nc.gpsimd.dma_start(tden[:], tflat.rearrange("(P f) -> P f", P=128))
            nc.gpsimd.dma_start(
                tsh[0:127, :],
                tflat[512 : 512 + 127 * 2048].rearrange("(P f) -> P f", P=127),
            )
            nc.gpsimd.dma_start(tsh[127:128, 0:1536], tflat[260608:262144][None, :])
            nc.gpsimd.dma_start(tshm[0:1, 512:2048], tflat[0:1536][None, :])
            nc.gpsimd.dma_start(
                tshm[1:128, :],
                tflat[1536 : 1536 + 127 * 2048].rearrange("(P f) -> P f", P=127),
            )

            # ---- x views ----
            xv = x_d.ap().rearrange("c (B n) -> c B n", n=8192)  # (21,32,8192)
            tvs = t_d.ap().rearrange("(B r) w -> B (r w)", r=16)  # (32,8192)

            # ---- gpsimd/SWDGE queue: every x doorbell before the cc trigger
            x_tiles = []
            x0 = xpool.tile([128, 8192], BF16, tag="x")
            for q in range(4):
                nc.gpsimd.dma_start(
                    x0[:, 2048 * q : 2048 * (q + 1)],
                    xv[0:4, :, 2048 * q : 2048 * (q + 1)],
                )
            x_tiles.append(x0)
            tb16 = singles.tile([128, 8192], BF16, tag="tb16")
            nc.gpsimd.dma_start(tb16[:], tvs[None, :, :].to_broadcast((4, 32, 8192)))
            for k in (1, 2, 3, 4):
                xk = xpool.tile([128, 8192], BF16, tag="x")
                nc.gpsimd.dma_start(xk[:], xv[4 * k : 4 * k + 4, :, :])
                x_tiles.append(xk)
            x21 = singles.tile([128, 2048], BF16, tag="x21")
            nc.gpsimd.dma_start(
                x21[:], x_d.ap()[20:21, :].rearrange("c (P n) -> (c P) n", n=2048)
            )

            # ---- boundary map on DVE (while x0 is in flight) ----
            # rd = (tden != tsh); rdm = (tshm != tden); dv = rd|rdm -> rd;
            # 3-tap horizontal OR -> rdm; zero borders of rdm; pack to fp8.
            rd = bm.tile([128, 2048], BF16, tag="rd")
            rdm = bm.tile([128, 2048], BF16, tag="rdm")
            nc.vector.tensor_tensor(rd[:], tden[:], tsh[:], op.not_equal)
            nc.vector.tensor_tensor(rdm[:], tshm[:], tden[:], op.not_equal)
            nc.vector.tensor_tensor(rd[:], rd[:], rdm[:], op.max)
            nc.vector.tensor_tensor(
                rdm[:, 1:2047], rd[:, 0:2046], rd[:, 1:2047], op.max
            )
            nc.vector.tensor_tensor(
                rdm[:, 1:2047], rdm[:, 1:2047], rd[:, 2:2048], op.max
            )
            rv = rdm[:].rearrange("P (r w) -> P r w", w=W)
            nc.vector.memset(rv[:, :, 0:1], 0.0)
            nc.vector.memset(rv[:, :, 511:512], 0.0)
            nc.vector.memset(rdm[0:1, 0:W], 0.0)
            nc.sync.dma_start(rdm[127:128, 3 * W : 4 * W], zrow[:])
            cc8 = bm.tile([128, 2048], FP8, tag="cc8")
            nc.vector.tensor_copy(cc8[:], rdm[:])

            # ---- collective: AllReduce(add) of the local map (fp8, 256KB)
            cc_in = dram.tile([H, W], FP8, tag="cc_in")
            cc_out = dram.tile([H, W], FP8, tag="cc_out")
            nc.sync.dma_start(
                cc_in[:].rearrange("(P r) w -> P (r w)", r=4), cc8[:]
            )
            if use_cc:
                nc.gpsimd.collective_compute(
                    "AllReduce",
                    op.add,
                    replica_groups=groups,
                    ins=[cc_in.opt()],
                    outs=[cc_out.opt()],
                )
            else:
                cc_out = cc_in

            # ---- weight image source (bd <- reduced map, on sync) ----
            bd = singles.tile([128, 2048], FP8, tag="bd")
            ccv = (
                cc_out[:]
                .rearrange("(B r) w -> B (r w)", r=16)
                .rearrange("B (q n) -> B q n", q=4)
            )
            for q in range(4):
                nc.sync.dma_start(bd[32 * q : 32 * q + 32, :], ccv[:, q, :])

            # ---- main loop: 5 chunks of 4 channels, processed in 4096-col
            # halves (k=0 in 2048-col quarters for a fast pipeline start) ----
            sums = psum.tile([128, 2048], F32, tag="sums")
            gath = psum.tile([128, 2048], F32, tag="gath")
            for k in range(5):
                x_t = x_tiles[k]
                npc = 4 if k == 0 else 2
                fpp = 8192 // npc
                for h in range(npc):
                    sl = slice(fpp * h, fpp * (h + 1))
                    ex = expool.tile([128, 4096], BF16, tag="ex")
                    mk = mkpool.tile([128, 4096], BF16, tag="mk")
                    exs = ex[:, 0:fpp]
                    mks = mk[:, 0:fpp]
                    nc.scalar.activation(exs, x_t[:, sl], Exp)
                    nc.vector.tensor_scalar(
                        mks, tb16[:, sl], cvec[:, k : k + 1], None, op.is_equal
                    )
                    nc.vector.tensor_tensor(mks, mks, x_t[:, sl], op.mult)
                    for wi in range(fpp // 2048):
                        w4 = (fpp // 2048) * h + wi
                        q0 = 32 * w4
                        for j in range(4):
                            fs = 2048 * wi + 512 * j
                            nc.tensor.matmul(
                                sums[q0 : q0 + 32, 512 * j : 512 * (j + 1)],
                                kxm[:, :],
                                exs[:, fs : fs + 512],
                                start=(k == 0),
                                stop=False,
                                tile_position=(0, q0),
                                skip_group_check=True,
                            )
                            nc.tensor.matmul(
                                gath[q0 : q0 + 32, 512 * j : 512 * (j + 1)],
                                kxm[:, :],
                                mks[:, fs : fs + 512],
                                start=(k == 0),
                                stop=False,
                                tile_position=(0, q0),
                                skip_group_check=True,
                            )

            # ---- channel 20 (flat [128,2048] layout, permutation stationary)
            ex21 = singles.tile([128, 2048], BF16, tag="ex21")
            mk21 = singles.tile([128, 2048], BF16, tag="mk21")
            nc.scalar.activation(ex21[:], x21[:], Exp)
            nc.vector.tensor_scalar(mk21[:], tden[:], 20.0, None, op.is_equal)
            nc.vector.tensor_tensor(mk21[:], mk21[:], x21[:], op.mult)

            # w / -w images (bd depends on the collective; emitted here so the
            # DVE only stalls on it right before the final phase)
            w16 = singles.tile([128, 2048], BF16, tag="w16")
            wneg = singles.tile([128, 2048], BF16, tag="wneg")
            nc.vector.tensor_scalar(w16[:], bd[:], 0.0, None, op.is_gt)
            nc.vector.tensor_scalar(wneg[:], w16[:], -2.0, -1.0, op.mult, op.add)
            nc.vector.tensor_scalar(w16[:], w16[:], 2.0, 1.0, op.mult, op.add)

            logs = singles.tile([128, 2048], BF16, tag="logs")
            partials = singles.tile([128, 4], F32, tag="partials")
            # last-chunk matmuls and the final phase, interleaved per half so
            # ln/stt of banks 0-1 overlap the matmuls of banks 2-3
            for half in range(2):
                js = (0, 1) if half == 0 else (2, 3)
                for j in js:
                    nc.tensor.matmul(
                        sums[:, 512 * j : 512 * (j + 1)],
                        perm[:, :],
                        ex21[:, 512 * j : 512 * (j + 1)],
                        start=False,
                        stop=True,
                        tile_position=(0, 0),
                        skip_group_check=True,
                    )
                for j in js:
                    nc.tensor.matmul(
                        gath[:, 512 * j : 512 * (j + 1)],
                        perm[:, :],
                        mk21[:, 512 * j : 512 * (j + 1)],
                        start=False,
                        stop=True,
                        tile_position=(0, 0),
                        skip_group_check=True,
                    )
                hs = slice(1024 * half, 1024 * (half + 1))
                nc.scalar.activation(logs[:, hs], sums[:, hs], Ln)
                wd = singles.tile([128, 1024], BF16, tag=f"wd{half}")
                wd2 = singles.tile([128, 1024], BF16, tag=f"wd2{half}")
                nc.vector.scalar_tensor_tensor(
                    wd[:], logs[:, hs], 1.0, w16[:, hs], op.mult, op.mult,
                    accum_out=partials[:, 2 * half : 2 * half + 1],
                )
                nc.vector.scalar_tensor_tensor(
                    wd2[:], gath[:, hs], 1.0, wneg[:, hs], op.mult, op.mult,
                    accum_out=partials[:, 2 * half + 1 : 2 * half + 2],
                )

            # ---- final reduction ----
            totp = psum.tile([1, 4], F32, tag="sums")
            nc.tensor.matmul(totp[:], ones[:], partials[:], start=True, stop=True)
            finr = singles.tile([1, 1], F32, tag="finr")
            nc.vector.reduce_sum(finr[:], totp[:], axis=mybir.AxisListType.X)
            fin = singles.tile([1, 1], F32, tag="fin")
            nc.scalar.activation(fin[:], finr[:], Copy, scale=1.0 / NTOT)
            nc.sync.dma_start(out_d[:], fin[:])

    nc.compile()
    return nc


_NC = None


def _get_nc():
    global _NC
    if _NC is None:
        _NC = build_nc()
    return _NC


def make_in_maps(inputs, targets):
    in_maps = []
    for i in range(NCORES):
        in_maps.append(
            {
                "x": np.ascontiguousarray(
                    np.asarray(inputs[i], dtype=np.float32)
                    .reshape(C, NPIX)
                    .astype(ml_dtypes.bfloat16)
                ),
                "t16": np.asarray(targets[i]).astype(ml_dtypes.bfloat16),
            }
        )
    return in_maps


def run_device(inputs, targets, trace=False):
    nc = _get_nc()
    res = bass_utils.run_bass_kernel_spmd(
        nc,
        make_in_maps(inputs, targets),
        core_ids=list(range(NCORES)),
        trace=trace,
    )
    return res


def kernel(inputs, targets):
    res = run_device(inputs, targets, trace=False)
    # each core returns its local weighted-sum / (B*H*W); the global mean is
    # the sum of the 8 partials (final reduction of the batch shard).
    return np.float32(sum(float(r["out"][0, 0]) for r in res.results))


# revision 8
# speedup vs baseline: 1.6066x; 1.1850x over previous
"""BoundaryLoss Trainium2 kernel (8 NeuronCores, data-parallel over batch).

Per core (one (21,512,512) image): ce[p] = ln(sum_c exp(x[c,p])) - x[t[p],p],
weighted by w[p] = 1 + 2*boundary[p] and summed; host sums 8 partials / BHW.

v4 plan (from the v1-v3 traces: SWDGE sustains only ~150GB/s here, each
gpsimd dma_start costs ~3.5us of queue time incl. drain, and the 8-rank
AllReduce is latency-bound at ~90-110us):
- x is fp8_e4m3 (host-cast; exp reads fp8 directly, ACT rate is dtype
  independent) halving the dominant stream to 5.5MB.  The x_t gather reads
  the bf16 EX tile instead of x (mask 4x-ts + 2x-tt stay in 2-byte mode);
  the tail takes ln(gath)=x_t, costing one extra Ln per half.  Host-checked
  rel err of fp8+exp-roundtrip: 1.5e-6.
- DMA spread over all three DGE paths: scalar ring x0+x3, sync ring x1,
  gpsimd t3+tb16+x2+x4+x21, so the fp8 stream lands well ahead of ACT.
- t ships as one host-prepped (128,6144) bf16 tensor: flat t | shift+512 |
  shift-512 with edges pre-zeroed -> one DMA, no zrow fixups for tsh/tshm.
- The collective trigger is emitted on gpsimd right after t3: it blocks that
  queue until cc_in lands (~16us) but fires the AllReduce as early as
  possible; later gpsimd doorbells simply queue behind it.  cc payload fp8
  (256KB), output addr_space=Shared (the documented fast path).
- Final phase: ln(sums)/ln(gath) per half; two scalar_tensor_tensor ops per
  half with accum_out produce sum(w*lnS) and sum(-w*ln gath) directly
  (w/-w images from the reduced map); ones-matmul + reduce + scaled copy.
"""

import sys

sys.path.insert(0, "/opt/trn_rl_repo")

import numpy as np
import ml_dtypes

import concourse.bass as bass
import concourse.bacc as bacc
import concourse.tile as tile
from concourse import mybir
from concourse import bass_utils

F32 = mybir.dt.float32
BF16 = mybir.dt.bfloat16
FP8 = mybir.dt.float8e4

C = 21          # channels
H = W = 512
NPIX = H * W    # 262144 pixels per core
NCORES = 8
NTOT = float(NCORES * NPIX)

Exp = mybir.ActivationFunctionType.Exp
Ln = mybir.ActivationFunctionType.Ln
Copy = mybir.ActivationFunctionType.Copy
op = mybir.AluOpType


def _consts():
    # kxm[p, m] = 1 if p % 32 == m: block-sum over the 4 channels packed per
    # chunk (partition p = c_local*32 + superblock).
    kxm = np.zeros((128, 32), np.float32)
    for p in range(128):
        kxm[p, p % 32] = 1.0
    # perm[p, m] = 1 iff m = 32*(p%4) + p//4: maps the flat-layout partition
    # p = sb*4 + w of channel 20 onto PSUM row 32*w + sb.
    perm = np.zeros((128, 128), np.float32)
    for p in range(128):
        perm[p, 32 * (p % 4) + p // 4] = 1.0
    # cvec[p, k] = absolute channel index of partition p in chunk k.
    cvec = np.zeros((128, 5), np.float32)
    for k in range(5):
        cvec[:, k] = 4 * k + np.arange(128) // 32
    return (
        kxm.astype(ml_dtypes.bfloat16),
        perm.astype(ml_dtypes.bfloat16),
        cvec,
    )


def build_nc(use_cc=True):
    nc = bacc.Bacc(
        "TRN2",
        target_bir_lowering=False,
        debug=False,
        num_devices=NCORES,
        num_swdge_queues=1,
        dynamic_dma_scratch_size=16384,
    )

    x_d = nc.dram_tensor("x", [C, NPIX], FP8, kind="ExternalInput")
    t_d = nc.dram_tensor("t16", [H, W], BF16, kind="ExternalInput")
    t3_d = nc.dram_tensor("t3", [128, 6144], BF16, kind="ExternalInput")
    out_d = nc.dram_tensor("out", [1, 1], F32, kind="ExternalOutput")

    kxm_np, perm_np, cvec_np = _consts()
    kxm_d = nc.inline_tensor(kxm_np, name="kxm")
    perm_d = nc.inline_tensor(perm_np, name="perm")
    cvec_d = nc.inline_tensor(cvec_np, name="cvec")
    ones_d = nc.inline_tensor(np.ones((128, 1), np.float32), name="ones")

    groups = [list(range(NCORES))]

    with tile.TileContext(nc) as tc:
        with (
            tc.tile_pool(name="singles", bufs=1) as singles,
            tc.tile_pool(name="xpool", bufs=5) as xpool,
            tc.tile_pool(name="expool", bufs=2) as expool,
            tc.tile_pool(name="mkpool", bufs=2) as mkpool,
            tc.tile_pool(name="bm", bufs=1) as bm,
            tc.tile_pool(name="psum", bufs=1, space="PSUM") as psum,
            tc.tile_pool(name="dram", bufs=1, space="DRAM") as dram,
        ):
            # ---- x views ----
            xv = x_d.ap().rearrange("c (B n) -> c B n", n=8192)  # (21,32,8192)
            tvs = t_d.ap().rearrange("(B r) w -> B (r w)", r=16)  # (32,8192)

            # ---- scalar/HWDGE ring: x0 halves + x3 (issued before any ACT)
            x_tiles = [
                xpool.tile([128, 8192], FP8, tag="x", name=f"xt{_k}")
                for _k in range(5)
            ]
            for hh in range(2):
                nc.scalar.dma_start(
                    x_tiles[0][:, 4096 * hh : 4096 * (hh + 1)],
                    xv[0:4, :, 4096 * hh : 4096 * (hh + 1)],
                )
            nc.scalar.dma_start(x_tiles[3][:], xv[12:16, :, :])

            # ---- sync/HWDGE ring: consts, x1, then cc_in store/bd/out later
            kxm = singles.tile([128, 32], BF16, tag="kxm")
            perm = singles.tile([128, 128], BF16, tag="perm")
            cvec = singles.tile([128, 5], F32, tag="cvec")
            ones = singles.tile([128, 1], F32, tag="ones")
            zrow = singles.tile([1, W], BF16, tag="zrow")
            nc.sync.dma_start(kxm[:], kxm_d[:])
            nc.sync.dma_start(perm[:], perm_d[:])
            nc.sync.dma_start(cvec[:], cvec_d[:])
            nc.sync.dma_start(ones[:], ones_d[:])
            nc.vector.memset(zrow[:], 0.0)
            nc.sync.dma_start(x_tiles[1][:], xv[4:8, :, :])

            # ---- gpsimd/SWDGE: t3 first, then the collective trigger, then
            # the remaining bulk (tb16 halves, x2, x4, x21)
            t3 = bm.tile([128, 6144], BF16, tag="t3")
            nc.gpsimd.dma_start(t3[:], t3_d[:])
            tden = t3[:, 0:2048]
            tsh = t3[:, 2048:4096]
            tshm = t3[:, 4096:6144]

            # boundary map on DVE (x0 still in flight)
            rd = bm.tile([128, 2048], BF16, tag="rd")
            rdm = bm.tile([128, 2048], BF16, tag="rdm")
            nc.vector.tensor_tensor(rd[:], tden, tsh, op.not_equal)
            nc.vector.tensor_tensor(rdm[:], tshm, tden, op.not_equal)
            nc.vector.tensor_tensor(rd[:], rd[:], rdm[:], op.max)
            nc.vector.tensor_tensor(
                rdm[:, 1:2047], rd[:, 0:2046], rd[:, 1:2047], op.max
            )
            nc.vector.tensor_tensor(
                rdm[:, 1:2047], rdm[:, 1:2047], rd[:, 2:2048], op.max
            )
            rv = rdm[:].rearrange("P (r w) -> P r w", w=W)
            nc.vector.memset(rv[:, :, 0:1], 0.0)
            nc.vector.memset(rv[:, :, 511:512], 0.0)
            nc.vector.memset(rdm[0:1, 0:W], 0.0)
            nc.sync.dma_start(rdm[127:128, 3 * W : 4 * W], zrow[:])
            cc8 = bm.tile([128, 2048], FP8, tag="cc8")
            nc.vector.tensor_copy(cc8[:], rdm[:])

            cc_in = dram.tile([H, W], FP8, tag="cc_in")
            cc_out = dram.tile([H, W], FP8, tag="cc_out", addr_space="Shared")
            nc.sync.dma_start(
                cc_in[:].rearrange("(P r) w -> P (r w)", r=4), cc8[:]
            )
            if use_cc:
                nc.gpsimd.collective_compute(
                    "AllReduce",
                    op.add,
                    replica_groups=groups,
                    ins=[cc_in.opt()],
                    outs=[cc_out.opt()],
                )
            else:
                cc_out = cc_in

            # remaining bulk on gpsimd (queued behind the cc trigger)
            tb16 = singles.tile([128, 8192], BF16, tag="tb16")
            for hh in range(2):
                nc.gpsimd.dma_start(
                    tb16[:, 4096 * hh : 4096 * (hh + 1)],
                    tvs[None, :, 4096 * hh : 4096 * (hh + 1)].to_broadcast(
                        (4, 32, 4096)
                    ),
                )
            nc.gpsimd.dma_start(x_tiles[2][:], xv[8:12, :, :])
            nc.gpsimd.dma_start(x_tiles[4][:], xv[16:20, :, :])
            x21 = singles.tile([128, 2048], FP8, tag="x21")
            nc.gpsimd.dma_start(
                x21[:], x_d.ap()[20:21, :].rearrange("c (P n) -> (c P) n", n=2048)
            )

            # ---- weight image source (bd <- reduced map, on sync) ----
            bd = singles.tile([128, 2048], FP8, tag="bd")
            ccv = (
                cc_out[:]
                .rearrange("(B r) w -> B (r w)", r=16)
                .rearrange("B (q n) -> B q n", q=4)
            )
            for q in range(4):
                nc.sync.dma_start(bd[32 * q : 32 * q + 32, :], ccv[:, q, :])

            # ---- main loop: 5 chunks of 4 channels, in 4096-col halves ----
            sums = psum.tile([128, 2048], F32, tag="sums")
            gath = psum.tile([128, 2048], F32, tag="gath")
            for k in range(5):
                x_t = x_tiles[k]
                for h in range(2):
                    sl = slice(4096 * h, 4096 * (h + 1))
                    ex = expool.tile([128, 4096], BF16, tag="ex")
                    mk = mkpool.tile([128, 4096], BF16, tag="mk")
                    nc.scalar.activation(ex[:], x_t[:, sl], Exp)
                    nc.vector.tensor_scalar(
                        mk[:], tb16[:, sl], cvec[:, k : k + 1], None, op.is_equal
                    )
                    nc.vector.tensor_tensor(mk[:], mk[:], ex[:], op.mult)
                    for wi in range(2):
                        w4 = 2 * h + wi
                        q0 = 32 * w4
                        for j in range(4):
                            fs = 2048 * wi + 512 * j
                            nc.tensor.matmul(
                                sums[q0 : q0 + 32, 512 * j : 512 * (j + 1)],
                                kxm[:, :],
                                ex[:, fs : fs + 512],
                                start=(k == 0),
                                stop=False,
                                tile_position=(0, q0),
                                skip_group_check=True,
                            )
                            nc.tensor.matmul(
                                gath[q0 : q0 + 32, 512 * j : 512 * (j + 1)],
                                kxm[:, :],
                                mk[:, fs : fs + 512],
                                start=(k == 0),
                                stop=False,
                                tile_position=(0, q0),
                                skip_group_check=True,
                            )

            # ---- channel 20 (flat [128,2048] layout, permutation stationary)
            ex21 = singles.tile([128, 2048], BF16, tag="ex21")
            mk21 = singles.tile([128, 2048], BF16, tag="mk21")
            nc.scalar.activation(ex21[:], x21[:], Exp)
            nc.vector.tensor_scalar(mk21[:], tden, 20.0, None, op.is_equal)
            nc.vector.tensor_tensor(mk21[:], mk21[:], ex21[:], op.mult)

            # w / -w images (bd depends on the collective; emitted here so the
            # DVE only stalls on it right before the final phase)
            w16 = singles.tile([128, 2048], BF16, tag="w16")
            wneg = singles.tile([128, 2048], BF16, tag="wneg")
            nc.vector.tensor_scalar(w16[:], bd[:], 0.0, None, op.is_gt)
            nc.vector.tensor_scalar(wneg[:], w16[:], -2.0, -1.0, op.mult, op.add)
            nc.vector.tensor_scalar(w16[:], w16[:], 2.0, 1.0, op.mult, op.add)

            logs = singles.tile([128, 2048], BF16, tag="logs")
            logs2 = singles.tile([128, 2048], BF16, tag="logs2")
            partials = singles.tile([128, 4], F32, tag="partials")
            # last-chunk matmuls and the final phase, interleaved per half so
            # ln/stt of banks 0-1 overlap the matmuls of banks 2-3
            for half in range(2):
                js = (0, 1) if half == 0 else (2, 3)
                for j in js:
                    nc.tensor.matmul(
                        sums[:, 512 * j : 512 * (j + 1)],
                        perm[:, :],
                        ex21[:, 512 * j : 512 * (j + 1)],
                        start=False,
                        stop=True,
                        tile_position=(0, 0),
                        skip_group_check=True,
                    )
                for j in js:
                    nc.tensor.matmul(
                        gath[:, 512 * j : 512 * (j + 1)],
                        perm[:, :],
                        mk21[:, 512 * j : 512 * (j + 1)],
                        start=False,
                        stop=True,
                        tile_position=(0, 0),
                        skip_group_check=True,
                    )
                hs = slice(1024 * half, 1024 * (half + 1))
                nc.scalar.activation(logs[:, hs], sums[:, hs], Ln)
                nc.scalar.activation(logs2[:, hs], gath[:, hs], Ln)
                wd = singles.tile([128, 1024], BF16, tag=f"wd{half}")
                wd2 = singles.tile([128, 1024], BF16, tag=f"wd2{half}")
                nc.vector.scalar_tensor_tensor(
                    wd[:], logs[:, hs], 1.0, w16[:, hs], op.mult, op.mult,
                    accum_out=partials[:, 2 * half : 2 * half + 1],
                )
                nc.vector.scalar_tensor_tensor(
                    wd2[:], logs2[:, hs], 1.0, wneg[:, hs], op.mult, op.mult,
                    accum_out=partials[:, 2 * half + 1 : 2 * half + 2],
                )

            # ---- final reduction ----
            totp = psum.tile([1, 4], F32, tag="sums")
            nc.tensor.matmul(totp[:], ones[:], partials[:], start=True, stop=True)
            finr = singles.tile([1, 1], F32, tag="finr")
            nc.vector.reduce_sum(finr[:], totp[:], axis=mybir.AxisListType.X)
            fin = singles.tile([1, 1], F32, tag="fin")
            nc.scalar.activation(fin[:], finr[:], Copy, scale=1.0 / NTOT)
            nc.sync.dma_start(out_d[:], fin[:])

    nc.compile()
    return nc


_NC = None


def _get_nc():
    global _NC
    if _NC is None:
        _NC = build_nc()
    return _NC


def _make_t3(t_u8_flat):
    # (128, 6144) bf16: [flat | shifted +512 (tail zeros) | shifted -512]
    f = t_u8_flat.astype(np.float32)
    tsh = np.zeros(NPIX, np.float32)
    tsh[: NPIX - 512] = f[512:]
    tshm = np.zeros(NPIX, np.float32)
    tshm[512:] = f[: NPIX - 512]
    t3 = np.concatenate(
        [f.reshape(128, 2048), tsh.reshape(128, 2048), tshm.reshape(128, 2048)],
        axis=1,
    )
    return np.ascontiguousarray(t3.astype(ml_dtypes.bfloat16))


def make_in_maps(inputs, targets):
    in_maps = []
    for i in range(NCORES):
        t_i = np.asarray(targets[i])
        in_maps.append(
            {
                "x": np.ascontiguousarray(
                    np.asarray(inputs[i], dtype=np.float32)
                    .reshape(C, NPIX)
                    .astype(ml_dtypes.float8_e4m3fn)
                ),
                "t16": np.ascontiguousarray(t_i.astype(ml_dtypes.bfloat16)),
                "t3": _make_t3(t_i.reshape(NPIX)),
            }
        )
    return in_maps


def run_device(inputs, targets, trace=False):
    nc = _get_nc()
    res = bass_utils.run_bass_kernel_spmd(
        nc,
        make_in_maps(inputs, targets),
        core_ids=list(range(NCORES)),
        trace=trace,
    )
    return res


def kernel(inputs, targets):
    res = run_device(inputs, targets, trace=False)
    # each core returns its local weighted-sum / (B*H*W); the global mean is
    # the sum of the 8 partials (final reduction of the batch shard).
    return np.float32(sum(float(r["out"][0, 0]) for r in res.results))
